# revision 1
# baseline (speedup 1.0000x reference)
"""Trainium2 Bass kernel: GQA attention layer with RoPE + int8 quant-dequant KV.

Tensor-parallel over heads across 8 NeuronCores: core c owns q-heads
[4c, 4c+4) and kv-head c.  Each core computes its partial output
y_c = attn_out_local @ wo_local.T; the host sums the 8 partials.

Per-core dataflow (all "T" tensors are [feature, token] with feature on
SBUF partitions):
  hsT --matmul(f32r)--> qT/kT/vT --RoPE(perm-matmul + DVE)-->
  --int8 quant-dequant (gpsimd absmax + magic-round on DVE)-->
  scores sT[k,q] = kT.T @ qT --exp(ACT)--> masked --> AV + ones-sum (PE)
  --> normalize --> aT(bf16) --wo matmul(bf16)--> y partial
"""
import math
import numpy as np
from contextlib import ExitStack

import concourse.bass as bass
import concourse.bacc as bacc
import concourse.mybir as mybir
import concourse.tile as tile
from concourse.bass_utils import run_bass_kernel_spmd
from concourse.masks import make_identity

F32 = mybir.dt.float32
F32R = mybir.dt.float32r
BF16 = mybir.dt.bfloat16
AF = mybir.ActivationFunctionType
ALU = mybir.AluOpType
AX = mybir.AxisListType

MAGIC = 1.5 * 2.0**23  # fp32 RNE integer-rounding magic constant
NCORES = 8


def build_nc(S=2048, D=4096, HL=4, QT=512, MMDT=BF16):
    """Build the per-core Bass graph. HL = local q heads (1 local kv head)."""
    DT = D // 128    # contraction tiles for projections
    NQ = S // QT     # query tiles
    DB = QT // 128   # 128-blocks per query tile
    KB = S // 128    # total k blocks
    NDC = D // 512   # wo output column tiles

    nc = bacc.Bacc("TRN2")
    hsT = nc.declare_dram_parameter("hsT", [D, S], MMDT, isOutput=False)
    wqT = nc.declare_dram_parameter("wqT", [D, HL * 128], MMDT, isOutput=False)
    wkT = nc.declare_dram_parameter("wkT", [D, 128], MMDT, isOutput=False)
    wvT = nc.declare_dram_parameter("wvT", [D, 128], MMDT, isOutput=False)
    woT = nc.declare_dram_parameter("woT", [HL * 128, D], BF16, isOutput=False)
    cosT = nc.declare_dram_parameter("cosT", [128, S], F32, isOutput=False)
    sinT = nc.declare_dram_parameter("sinT", [128, S], F32, isOutput=False)
    mks = nc.declare_dram_parameter("mks", [DB, 128, QT], F32, isOutput=False)
    rotT = nc.declare_dram_parameter("rotT", [128, 128], MMDT, isOutput=False)
    y = nc.declare_dram_parameter("y", [S, D], F32, isOutput=True)

    with tile.TileContext(nc) as tc, ExitStack() as ctx:
        const = ctx.enter_context(tc.tile_pool(name="const", bufs=1))
        persist = ctx.enter_context(tc.tile_pool(name="persist", bufs=1))
        hs_pool = ctx.enter_context(tc.tile_pool(name="hs", bufs=4))
        wq_pool = ctx.enter_context(tc.tile_pool(name="wqp", bufs=4))
        wkv_pool = ctx.enter_context(tc.tile_pool(name="wkvp", bufs=4))
        work = ctx.enter_context(tc.tile_pool(name="work", bufs=6))
        work128 = ctx.enter_context(tc.tile_pool(name="work128", bufs=6))
        expp = ctx.enter_context(tc.tile_pool(name="expp", bufs=4))
        qpool = ctx.enter_context(tc.tile_pool(name="qpool", bufs=2 * HL))
        apool = ctx.enter_context(tc.tile_pool(name="apool", bufs=2 * HL))
        ypool = ctx.enter_context(tc.tile_pool(name="ypool", bufs=4))
        rows = ctx.enter_context(tc.tile_pool(name="rows", bufs=8))
        pbig = ctx.enter_context(tc.tile_pool(name="pbig", bufs=7, space="PSUM"))
        psum1 = ctx.enter_context(tc.tile_pool(name="psum1", bufs=1, space="PSUM"))
        drampool = ctx.enter_context(tc.tile_pool(name="drampool", bufs=4, space="DRAM"))

        # ---- constants ----
        cos_sb = const.tile([128, S], F32, name="cos", tag="cos")
        nc.sync.dma_start(out=cos_sb[:], in_=cosT[:])
        sin_sb = const.tile([128, S], F32, name="sin", tag="sin")
        nc.sync.dma_start(out=sin_sb[:], in_=sinT[:])
        mks_sb = []
        for r in range(DB):
            m = const.tile([128, QT], F32, name=f"mk{r}", tag=f"mk{r}")
            nc.sync.dma_start(out=m[:], in_=mks[r, :, :])
            mks_sb.append(m)
        rot_sb = const.tile([128, 128], MMDT, name="rot", tag="rot")
        nc.sync.dma_start(out=rot_sb[:], in_=rotT[:])
        ident = const.tile([128, 128], F32, name="ident", tag="ident")
        make_identity(nc, ident[:])
        ones_col = const.tile([128, 1], MMDT, name="onec", tag="onec")
        nc.vector.memset(ones_col[:], 1.0)
        ones_row = const.tile([1, 128], MMDT, name="oner", tag="oner")
        nc.vector.memset(ones_row[:], 1.0)
        zbias = const.tile([128, 1], F32, name="zbias", tag="zbias")
        nc.vector.memset(zbias[:], 0.0)

        kT_all = persist.tile([128, S], MMDT, name="kT", tag="kT")
        v_nat = persist.tile([128, KB, 128], MMDT, name="vnat", tag="vnat")
        woT_sb = []
        for hb in range(HL):
            w = persist.tile([128, D], BF16, name=f"wo{hb}", tag=f"wo{hb}")
            nc.sync.dma_start(out=w[:], in_=woT[hb * 128:(hb + 1) * 128, :])
            woT_sb.append(w)

        def qd_nat_block(x_ap, out_ap):
            """int8 quant-dequant of one [tok(part), dh(free)] 128x128 block.

            absmax over the free (dh) axis per token, symmetric 127-step
            grid, round-to-nearest-even via the fp32 magic trick.
            """
            amax = rows.tile([128, 1], F32, name="row", tag="row")
            nc.vector.tensor_reduce(out=amax[:], in_=x_ap, axis=AX.X,
                                    op=ALU.max, apply_absolute_value=True)
            scl = rows.tile([128, 1], F32, name="row", tag="row")
            nc.vector.tensor_scalar(out=scl[:], in0=amax[:],
                                    scalar1=1.0 / 127.0, scalar2=1e-8,
                                    op0=ALU.mult, op1=ALU.max)
            inv = rows.tile([128, 1], F32, name="row", tag="row")
            nc.vector.reciprocal(inv[:], scl[:])
            xs = work128.tile([128, 128], F32, name="w128", tag="w128")
            nc.vector.tensor_scalar(out=xs[:], in0=x_ap, scalar1=inv[:],
                                    scalar2=None, op0=ALU.mult)
            nc.vector.tensor_scalar(out=xs[:], in0=xs[:], scalar1=MAGIC,
                                    scalar2=MAGIC, op0=ALU.add,
                                    op1=ALU.subtract)
            nc.vector.tensor_scalar(out=out_ap, in0=xs[:], scalar1=scl[:],
                                    scalar2=None, op0=ALU.mult)

        def rope(psum_in, cos_sl, sin_sl, out_ap):
            """RoPE in [feat, tok] layout; rotate-half via permutation matmul."""
            raw = work.tile([128, QT], MMDT, name="rawmm", tag="rawmm")
            nc.vector.tensor_copy(raw[:], psum_in[:])
            rot_ps = pbig.tile([128, QT], F32, name="big", tag="big")
            nc.tensor.matmul(rot_ps[:], rot_sb[:],
                             raw[:], start=True, stop=True)
            tmp = work.tile([128, QT], F32, name="work", tag="work")
            nc.vector.tensor_tensor(out=tmp[:], in0=raw[:], in1=cos_sl,
                                    op=ALU.mult)
            t2 = work.tile([128, QT], F32, name="work", tag="work")
            nc.vector.tensor_tensor(out=t2[:], in0=rot_ps[:], in1=sin_sl,
                                    op=ALU.mult)
            nc.vector.tensor_tensor(out=out_ap, in0=tmp[:], in1=t2[:],
                                    op=ALU.add)

        for I in range(NQ):
            qsl = slice(I * QT, (I + 1) * QT)
            cos_sl = cos_sb[:, qsl]
            sin_sl = sin_sb[:, qsl]

            # ---- q/k/v projections for this token tile ----
            pq = [pbig.tile([128, QT], F32, name="big", tag="big") for _ in range(HL)]
            pk = pbig.tile([128, QT], F32, name="big", tag="big")
            pv = pbig.tile([128, QT], F32, name="big", tag="big")
            for d in range(DT):
                dsl = slice(d * 128, (d + 1) * 128)
                hs_t = hs_pool.tile([128, QT], MMDT, name="hs", tag="hs")
                nc.sync.dma_start(out=hs_t[:], in_=hsT[dsl, qsl])
                wq_t = wq_pool.tile([128, HL * 128], MMDT, name="wq", tag="wq")
                nc.sync.dma_start(out=wq_t[:], in_=wqT[dsl, :])
                wk_t = wkv_pool.tile([128, 128], MMDT, name="wkv", tag="wkv")
                nc.sync.dma_start(out=wk_t[:], in_=wkT[dsl, :])
                wv_t = wkv_pool.tile([128, 128], MMDT, name="wkv", tag="wkv")
                nc.sync.dma_start(out=wv_t[:], in_=wvT[dsl, :])
                first, last = d == 0, d == DT - 1
                for h in range(HL):
                    nc.tensor.matmul(pq[h][:],
                                     wq_t[:, h * 128:(h + 1) * 128],
                                     hs_t[:],
                                     start=first, stop=last)
                nc.tensor.matmul(pk[:], wk_t[:],
                                 hs_t[:], start=first, stop=last)
                nc.tensor.matmul(pv[:], wv_t[:],
                                 hs_t[:], start=first, stop=last)

            # ---- RoPE q ----
            qts = []
            for h in range(HL):
                qt_t = qpool.tile([128, QT], MMDT, name="qt", tag="qt")
                rope(pq[h], cos_sl, sin_sl, qt_t[:])
                qts.append(qt_t)

            # ---- RoPE k; per 128-block: transpose -> qd -> transpose back ----
            krope = work.tile([128, QT], F32, name="work", tag="work")
            rope(pk, cos_sl, sin_sl, krope[:])
            for t in range(DB):
                t_sl = slice(t * 128, (t + 1) * 128)
                tr_ps = pbig.tile([128, 128], F32, name="big", tag="big")
                nc.tensor.transpose(tr_ps[:], krope[:, t_sl], ident[:])
                k_nat = work128.tile([128, 128], F32, name="w128", tag="w128")
                nc.vector.tensor_copy(k_nat[:], tr_ps[:])
                kq_nat = work128.tile([128, 128], F32, name="w128", tag="w128")
                qd_nat_block(k_nat[:], kq_nat[:])
                tr2_ps = pbig.tile([128, 128], F32, name="big", tag="big")
                nc.tensor.transpose(tr2_ps[:], kq_nat[:], ident[:])
                nc.vector.tensor_copy(kT_all[:, I * QT + t * 128:
                                              I * QT + (t + 1) * 128], tr2_ps[:])

            # ---- v: transpose to natural [tok, dh] blocks, then qd ----
            vraw = work.tile([128, QT], F32, name="work", tag="work")
            nc.vector.tensor_copy(vraw[:], pv[:])
            for t in range(DB):
                t_sl = slice(t * 128, (t + 1) * 128)
                tr_ps = pbig.tile([128, 128], F32, name="big", tag="big")
                nc.tensor.transpose(tr_ps[:], vraw[:, t_sl], ident[:])
                v_nat_raw = work128.tile([128, 128], F32, name="w128", tag="w128")
                nc.vector.tensor_copy(v_nat_raw[:], tr_ps[:])
                qd_nat_block(v_nat_raw[:], v_nat[:, I * DB + t, :])

            # ---- attention (causal, unnormalized exp + ones-sum) ----
            ats = []
            nkb = (I + 1) * DB
            for h in range(HL):
                out_ps = pbig.tile([128, QT], F32, name="big", tag="big")
                sum_ps = psum1.tile([1, QT], F32, name="sum", tag="sum")
                for j in range(nkb):
                    s_ps = pbig.tile([128, QT], F32, name="big", tag="big")
                    nc.tensor.matmul(s_ps[:],
                                     kT_all[:, j * 128:(j + 1) * 128],
                                     qts[h][:],
                                     start=True, stop=True)
                    e_sb = expp.tile([128, QT], MMDT, name="exp", tag="exp")
                    nc.scalar.activation(e_sb[:], s_ps[:], AF.Exp,
                                         bias=zbias[:],
                                         scale=1.0 / math.sqrt(128.0))
                    r = j - I * DB
                    if r >= 0:
                        nc.vector.tensor_tensor(out=e_sb[:], in0=e_sb[:],
                                                in1=mks_sb[r][:], op=ALU.mult)
                    first, last = j == 0, j == nkb - 1
                    nc.tensor.matmul(out_ps[:], v_nat[:, j, :],
                                     e_sb[:],
                                     start=first, stop=last)
                    nc.tensor.matmul(sum_ps[:], ones_col[:],
                                     e_sb[:],
                                     start=first, stop=last)
                rec = rows.tile([1, QT], F32, name="rec", tag="rec")
                nc.vector.reciprocal(rec[:], sum_ps[:])
                rec_d = drampool.tile([1, QT], F32, name="recd", tag="recd")
                nc.sync.dma_start(out=rec_d[:], in_=rec[:])
                brec = work.tile([128, QT], F32, name="work", tag="work")
                rec_bcast = bass.AP(
                    tensor=rec_d.tensor, offset=rec_d.offset,
                    ap=[[0, 128]] + list(rec_d.ap[1:]))
                nc.sync.dma_start(out=brec[:], in_=rec_bcast)
                a_t = apool.tile([128, QT], BF16, name="at", tag="at")
                nc.vector.tensor_tensor(out=a_t[:], in0=out_ps[:],
                                        in1=brec[:], op=ALU.mult)
                ats.append(a_t)

            # ---- wo partial: y[tok, dout] += aT.T @ woT ----
            for t in range(DB):
                t_sl = slice(t * 128, (t + 1) * 128)
                for dc in range(NDC):
                    y_ps = pbig.tile([128, 512], F32, name="big", tag="big")
                    for hb in range(HL):
                        nc.tensor.matmul(y_ps[:], ats[hb][:, t_sl],
                                         woT_sb[hb][:, dc * 512:(dc + 1) * 512],
                                         start=(hb == 0), stop=(hb == HL - 1))
                    y_sb = ypool.tile([128, 512], F32, name="y", tag="y")
                    nc.vector.tensor_copy(y_sb[:], y_ps[:])
                    nc.sync.dma_start(
                        out=y[I * QT + t * 128:I * QT + (t + 1) * 128,
                              dc * 512:(dc + 1) * 512],
                        in_=y_sb[:])
    nc.compile()
    return nc


def host_inputs(hidden_states, wq, wk, wv, wo, position_ids,
                S=2048, D=4096, HL=4, QT=512, ncores=NCORES, mmdt="bf16"):
    """Shard + preprocess inputs -> per-core in_maps."""
    import ml_dtypes
    cast = ((lambda a: np.ascontiguousarray(a).astype(ml_dtypes.bfloat16))
            if mmdt == "bf16" else (lambda a: np.ascontiguousarray(a)))
    DB = QT // 128
    hs = np.asarray(hidden_states, np.float32)[0]
    hsT = np.ascontiguousarray(hs.T)  # [D, S]

    pos = np.asarray(position_ids)[0].astype(np.float32)
    inv_freq = (1.0 / (10000.0 ** (np.arange(0, 128, 2, dtype=np.float32) / 128.0)))
    freqs = pos[:, None] * inv_freq[None, :]          # [S, 64]
    emb = np.concatenate([freqs, freqs], axis=1)      # [S, 128]
    cosT = np.ascontiguousarray(np.cos(emb).T).astype(np.float32)
    sinT = np.ascontiguousarray(np.sin(emb).T).astype(np.float32)

    kk = np.arange(128)[:, None]
    qq = np.arange(QT)[None, :]
    mks = np.stack([(kk + 128 * r <= qq) for r in range(DB)]).astype(np.float32)

    rotT = np.zeros((128, 128), np.float32)
    idx = np.arange(64)
    rotT[idx, idx + 64] = 1.0
    rotT[idx + 64, idx] = -1.0
    rotT = cast(rotT)

    wq = np.asarray(wq, np.float32)
    wk = np.asarray(wk, np.float32)
    wv = np.asarray(wv, np.float32)
    wo = np.asarray(wo, np.float32)

    hsT = cast(hsT)
    in_maps = []
    qh = HL * 128
    for c in range(ncores):
        wqT_c = cast(wq[c * qh:(c + 1) * qh, :].T)
        wkT_c = cast(wk[c * 128:(c + 1) * 128, :].T)
        wvT_c = cast(wv[c * 128:(c + 1) * 128, :].T)
        woT_c = np.ascontiguousarray(wo[:, c * qh:(c + 1) * qh].T).astype(
            ml_dtypes.bfloat16)
        in_maps.append({
            "hsT": hsT, "wqT": wqT_c, "wkT": wkT_c, "wvT": wvT_c,
            "woT": woT_c, "cosT": cosT, "sinT": sinT, "mks": mks,
            "rotT": rotT,
        })
    return in_maps


_NC_CACHE = {}
COMPUTE = "bf16"  # "bf16" or "f32r"


def kernel(hidden_states, wq, wk, wv, wo, position_ids):
    B, S, D = hidden_states.shape
    in_maps = host_inputs(hidden_states, wq, wk, wv, wo, position_ids,
                          S=S, D=D, mmdt=COMPUTE)
    key = (S, D, COMPUTE)
    if key not in _NC_CACHE:
        _NC_CACHE[key] = build_nc(S=S, D=D,
                                  MMDT=BF16 if COMPUTE == "bf16" else F32R)
    nc = _NC_CACHE[key]
    res = run_bass_kernel_spmd(nc, in_maps, core_ids=list(range(NCORES)),
                               trace=False)
    y = np.zeros((S, D), np.float64)
    for c in range(NCORES):
        y += res.results[c]["y"].astype(np.float64)
    return y.astype(np.float32)[None]



# revision 2
# speedup vs baseline: 1.2760x; 1.2760x over previous
"""Trainium2 Bass kernel v2: GQA attention + RoPE + int8 quant-dequant KV.

Tensor-parallel over heads across 8 NeuronCores: core c owns q-heads
[4c, 4c+4) and kv-head c.  Each core computes its partial output
y_c = attn_out_local @ wo_local.T in bf16; the host sums the 8 partials.

v2 structure (vs v1): batched DMAs (weights resident, ~50 DMA insts
total), software-pipelined PE stream (proj(I) | attn(I-1) | wo(I-1)
with rope/quant on DVE/ACT in the gaps), exp double-buffered 2 deep,
causal diagonal trimmed, sums as end-of-head PE pass, y staged bf16.
"""
import math
import numpy as np
from contextlib import ExitStack

import concourse.bass as bass
import concourse.bacc as bacc
import concourse.mybir as mybir
import concourse.tile as tile
from concourse.bass_utils import run_bass_kernel_spmd
from concourse.masks import make_identity

F32 = mybir.dt.float32
BF16 = mybir.dt.bfloat16
AF = mybir.ActivationFunctionType
ALU = mybir.AluOpType
AX = mybir.AxisListType

MAGIC = 1.5 * 2.0**23  # fp32 RNE integer-rounding magic constant
NCORES = 8
ROPE_MODE = "halves"   # "halves" (partition-offset DVE) or "matmul"


def build_nc(S=2048, D=4096, HL=4, QT=512, rope_mode=ROPE_MODE):
    DT = D // 128    # 32 contraction tiles
    NQ = S // QT     # 4 query tiles
    DB = QT // 128   # 4 128-blocks per query tile
    KB = S // 128    # 16 k blocks
    QH = HL * 128    # local q features
    ISQ = 1.0 / math.sqrt(128.0)

    nc = bacc.Bacc("TRN2")
    hsT = nc.declare_dram_parameter("hsT", [D, S], BF16, isOutput=False)
    wqT = nc.declare_dram_parameter("wqT", [D, QH], BF16, isOutput=False)
    wkT = nc.declare_dram_parameter("wkT", [D, 128], BF16, isOutput=False)
    wvT = nc.declare_dram_parameter("wvT", [D, 128], BF16, isOutput=False)
    woT = nc.declare_dram_parameter("woT", [QH, D], BF16, isOutput=False)
    cosT = nc.declare_dram_parameter("cosT", [128, S], BF16, isOutput=False)
    sinT = nc.declare_dram_parameter("sinT", [128, S], BF16, isOutput=False)
    mkT = nc.declare_dram_parameter("mkT", [128, 128], BF16, isOutput=False)
    rotT = nc.declare_dram_parameter("rotT", [128, 128], BF16, isOutput=False)
    y = nc.declare_dram_parameter("y", [S, D], BF16, isOutput=True)

    def packed3(param, inner):
        """DRAM [DT*128, inner] viewed as [128(p), DT, inner]."""
        ap = param[:]
        return bass.AP(tensor=ap.tensor, offset=0,
                       ap=[[inner, 128], [128 * inner, DT], [1, inner]])

    with tile.TileContext(nc) as tc, ExitStack() as ctx:
        const = ctx.enter_context(tc.tile_pool(name="const", bufs=1))
        persist = ctx.enter_context(tc.tile_pool(name="persist", bufs=1))
        hs_pool = ctx.enter_context(tc.tile_pool(name="hs", bufs=5))
        qpool = ctx.enter_context(tc.tile_pool(name="qpool", bufs=2 * HL))
        rawp = ctx.enter_context(tc.tile_pool(name="rawp", bufs=5))
        t14 = ctx.enter_context(tc.tile_pool(name="t14", bufs=4))
        krp = ctx.enter_context(tc.tile_pool(name="krp", bufs=2))
        w128 = ctx.enter_context(tc.tile_pool(name="w128", bufs=12))
        rows = ctx.enter_context(tc.tile_pool(name="rows", bufs=8))
        expp = ctx.enter_context(tc.tile_pool(name="expp", bufs=6))
        apool = ctx.enter_context(tc.tile_pool(name="apool", bufs=HL + 2))
        recp = ctx.enter_context(tc.tile_pool(name="recp", bufs=2))
        recbc = ctx.enter_context(tc.tile_pool(name="recbc", bufs=2))
        ysbp = ctx.enter_context(tc.tile_pool(name="ysb", bufs=2))
        wrk = ctx.enter_context(tc.tile_pool(name="wrk", bufs=2))
        psA = ctx.enter_context(tc.tile_pool(name="psA", bufs=3, space="PSUM"))
        psB = ctx.enter_context(tc.tile_pool(name="psB", bufs=3, space="PSUM"))
        psY = ctx.enter_context(tc.tile_pool(name="psY", bufs=2, space="PSUM"))

        # ---- persistent loads, ordered so proj(0) can start ASAP:
        # wq halves interleaved with hs(0) chunks, then k/v/rope consts,
        # wo (needed ~85us in) last.
        hs0_tiles = []

        def emit_hs_chunk(I, c):
            t = hs_pool.tile([128, 8, QT], BF16, name=f"hs{I}_{c}", tag="hs")
            hs_ap = hsT[:]
            src = bass.AP(tensor=hs_ap.tensor,
                          offset=(c * 8 * 128) * S + I * QT,
                          ap=[[S, 128], [128 * S, 8], [1, QT]])
            nc.sync.dma_start(out=t[:], in_=src)
            return t

        wq_sb = persist.tile([128, DT, QH], BF16, name="wq", tag="wq")
        wq_ap = wqT[:]
        nc.sync.dma_start(out=wq_sb[:, 0:DT // 2, :], in_=bass.AP(
            tensor=wq_ap.tensor, offset=0,
            ap=[[QH, 128], [128 * QH, DT // 2], [1, QH]]))
        hs0_tiles.append(emit_hs_chunk(0, 0))
        hs0_tiles.append(emit_hs_chunk(0, 1))
        nc.sync.dma_start(out=wq_sb[:, DT // 2:DT, :], in_=bass.AP(
            tensor=wq_ap.tensor, offset=(DT // 2) * 128 * QH,
            ap=[[QH, 128], [128 * QH, DT // 2], [1, QH]]))
        hs0_tiles.append(emit_hs_chunk(0, 2))
        hs0_tiles.append(emit_hs_chunk(0, 3))
        wk_sb = persist.tile([128, DT, 128], BF16, name="wk", tag="wk")
        nc.sync.dma_start(out=wk_sb[:], in_=packed3(wkT, 128))
        wv_sb = persist.tile([128, DT, 128], BF16, name="wv", tag="wv")
        nc.sync.dma_start(out=wv_sb[:], in_=packed3(wvT, 128))
        cos_sb = persist.tile([128, S], BF16, name="cos", tag="cos")
        nc.sync.dma_start(out=cos_sb[:], in_=cosT[:])
        sin_sb = persist.tile([128, S], BF16, name="sin", tag="sin")
        nc.sync.dma_start(out=sin_sb[:], in_=sinT[:])
        mk_sb = const.tile([128, 128], BF16, name="mk", tag="mk")
        nc.sync.dma_start(out=mk_sb[:], in_=mkT[:])
        rot_sb = const.tile([128, 128], BF16, name="rot", tag="rot")
        nc.sync.dma_start(out=rot_sb[:], in_=rotT[:])
        wo_sb = persist.tile([128, HL, D], BF16, name="wo", tag="wo")
        wo_ap = woT[:]
        nc.sync.dma_start(out=wo_sb[:], in_=bass.AP(
            tensor=wo_ap.tensor, offset=0,
            ap=[[D, 128], [128 * D, HL], [1, D]]))
        ident = const.tile([128, 128], BF16, name="ident", tag="ident")
        make_identity(nc, ident[:])
        ones_col = const.tile([128, 1], BF16, name="onec", tag="onec")
        nc.vector.memset(ones_col[:], 1.0)

        kT_all = persist.tile([128, S], BF16, name="kT", tag="kT")
        v_nat = persist.tile([128, KB, 128], BF16, name="vnat", tag="vnat")

        # ---- helpers ----
        def emit_qd(x_sb, out_ap):
            """int8 quant-dequant of one natural [tok, dh] 128x128 block."""
            amax = rows.tile([128, 1], F32, name="row", tag="row")
            nc.vector.tensor_reduce(out=amax[:], in_=x_sb, axis=AX.X,
                                    op=ALU.max, apply_absolute_value=True)
            scl = rows.tile([128, 1], F32, name="row", tag="row")
            nc.vector.tensor_scalar(out=scl[:], in0=amax[:],
                                    scalar1=1.0 / 127.0, scalar2=1e-8,
                                    op0=ALU.mult, op1=ALU.max)
            inv = rows.tile([128, 1], F32, name="row", tag="row")
            nc.vector.reciprocal(inv[:], scl[:])
            xs = w128.tile([128, 128], F32, name="xs", tag="w128")
            nc.vector.tensor_scalar(out=xs[:], in0=x_sb, scalar1=inv[:],
                                    scalar2=None, op0=ALU.mult)
            nc.vector.tensor_scalar(out=xs[:], in0=xs[:], scalar1=MAGIC,
                                    scalar2=MAGIC, op0=ALU.add,
                                    op1=ALU.subtract)
            nc.vector.tensor_scalar(out=out_ap, in0=xs[:], scalar1=scl[:],
                                    scalar2=None, op0=ALU.mult)

        def emit_rope(raw, qsl, out_ap):
            """RoPE in [feat, tok] layout."""
            if rope_mode == "halves":
                # cos/sin rows repeat (cos[p] == cos[p+64]), so every
                # two-SBUF-input op can be base-partition aligned.
                t1 = t14.tile([64, QT], F32, name="t1", tag="t14")
                nc.vector.tensor_tensor(out=t1[:], in0=raw[0:64, :],
                                        in1=cos_sb[0:64, qsl], op=ALU.mult)
                t2 = t14.tile([64, QT], F32, name="t2", tag="t14")
                nc.vector.tensor_tensor(out=t2[:], in0=raw[64:128, :],
                                        in1=sin_sb[64:128, qsl], op=ALU.mult)
                nc.vector.tensor_tensor(out=out_ap[0:64, :], in0=t1[:],
                                        in1=t2[:], op=ALU.subtract)
                t3 = t14.tile([64, QT], F32, name="t3", tag="t14")
                nc.vector.tensor_tensor(out=t3[:], in0=raw[64:128, :],
                                        in1=cos_sb[64:128, qsl], op=ALU.mult)
                t4 = t14.tile([64, QT], F32, name="t4", tag="t14")
                nc.vector.tensor_tensor(out=t4[:], in0=raw[0:64, :],
                                        in1=sin_sb[0:64, qsl], op=ALU.mult)
                nc.vector.tensor_tensor(out=out_ap[64:128, :], in0=t3[:],
                                        in1=t4[:], op=ALU.add)
            else:
                rot_ps = psB.tile([128, QT], F32, name="rotps", tag="ps")
                nc.tensor.matmul(rot_ps[:], rot_sb[:], raw, start=True,
                                 stop=True)
                w1 = wrk.tile([128, QT], F32, name="w1", tag="wrk")
                nc.vector.tensor_tensor(out=w1[:], in0=raw,
                                        in1=cos_sb[:, qsl], op=ALU.mult)
                w2 = wrk.tile([128, QT], F32, name="w2", tag="wrk")
                nc.vector.tensor_tensor(out=w2[:], in0=rot_ps[:],
                                        in1=sin_sb[:, qsl], op=ALU.mult)
                nc.vector.tensor_tensor(out=out_ap, in0=w1[:], in1=w2[:],
                                        op=ALU.add)

        # per-I state carried across the pipeline
        kq_tiles = [None] * NQ   # 4 quantized-k natural blocks per I
        vraw_tiles = [None] * NQ
        krope_tiles = [None] * NQ
        qts_tiles = [None] * NQ  # 4 roped q tiles per I
        at_tiles = [None] * NQ   # 4 normalized attention tiles per I

        def emit_hs_dma(I):
            if I == 0:
                return hs0_tiles
            return [emit_hs_chunk(I, c) for c in range(4)]

        def hs_slice(tiles, d):
            return tiles[d // 8][:, d % 8, :]

        def emit_proj_pass(I, hs_tiles, specs):
            """specs: list of (kind, idx) with kind in {q,k,v}; one psA
            accumulator each. Returns raw SBUF bf16 tiles (evacuated)."""
            accs = []
            for kind, idx in specs:
                accs.append(psA.tile([128, QT], F32, name=f"p{kind}{idx}",
                                     tag="psA"))
            for d in range(DT):
                h = hs_slice(hs_tiles, d)
                for (kind, idx), acc in zip(specs, accs):
                    if kind == "q":
                        w = wq_sb[:, d, idx * 128:(idx + 1) * 128]
                    elif kind == "k":
                        w = wk_sb[:, d, :]
                    else:
                        w = wv_sb[:, d, :]
                    nc.tensor.matmul(acc[:], w, h, start=(d == 0),
                                     stop=(d == DT - 1))
            raws = []
            for i, ((kind, idx), acc) in enumerate(zip(specs, accs)):
                r = rawp.tile([128, QT], BF16, name=f"raw{kind}{idx}",
                              tag="raw")
                if i % 2 == 0:
                    nc.scalar.activation(r[:], acc[:], AF.Copy, bias=0.0,
                                         scale=1.0)
                else:
                    nc.vector.tensor_copy(r[:], acc[:])
                raws.append(r)
            return raws

        def emit_proj(I, hs_tiles):
            qsl = slice(I * QT, (I + 1) * QT)
            raws_a = emit_proj_pass(I, hs_tiles, [("q", 0), ("q", 1)])
            raws_b = emit_proj_pass(I, hs_tiles, [("q", 2), ("q", 3)])
            raws_c = emit_proj_pass(I, hs_tiles, [("k", 0), ("v", 0)])
            # rope q (DVE work; runs during attn(I-1))
            qts = []
            for h, raw in enumerate(raws_a + raws_b):
                qt = qpool.tile([128, QT], BF16, name=f"qt{h}", tag="qt")
                emit_rope(raw[:], qsl, qt[:])
                qts.append(qt)
            qts_tiles[I] = qts
            kr = krp.tile([128, QT], BF16, name="krope", tag="krope")
            emit_rope(raws_c[0][:], qsl, kr[:])
            krope_tiles[I] = kr
            vraw_tiles[I] = raws_c[1]

        def emit_transp(I):
            """k/v transposes to natural layout + int8 qd (end of iter I)."""
            kqs = []
            for t in range(DB):
                t_sl = slice(t * 128, (t + 1) * 128)
                ktr = psA.tile([128, 128], BF16, name="ktr", tag="psA")
                nc.tensor.transpose(ktr[:], krope_tiles[I][:, t_sl], ident[:])
                kn = w128.tile([128, 128], BF16, name="kn", tag="w128")
                nc.vector.tensor_copy(kn[:], ktr[:])
                kq = w128.tile([128, 128], BF16, name="kq", tag="w128")
                emit_qd(kn[:], kq[:])
                kqs.append(kq)
            kq_tiles[I] = kqs
            for t in range(DB):
                t_sl = slice(t * 128, (t + 1) * 128)
                vtr = psA.tile([128, 128], BF16, name="vtr", tag="psA")
                nc.tensor.transpose(vtr[:], vraw_tiles[I][:, t_sl], ident[:])
                vn = w128.tile([128, 128], BF16, name="vn", tag="w128")
                nc.vector.tensor_copy(vn[:], vtr[:])
                emit_qd(vn[:], v_nat[:, I * DB + t, :])

        def emit_tbk(I):
            """transpose quantized k back to [dh, tok] into kT_all."""
            for t in range(DB):
                ktr2 = psA.tile([128, 128], BF16, name="ktr2", tag="psA")
                nc.tensor.transpose(ktr2[:], kq_tiles[I][t][:], ident[:])
                nc.vector.tensor_copy(
                    kT_all[:, I * QT + t * 128:I * QT + (t + 1) * 128],
                    ktr2[:])

        def emit_attn(I):
            """attention for query tile I (kv 0..(I+1)*DB-1), diag-trimmed."""
            js = []
            for j in range(I * DB):
                js.append((j, 0, QT))                      # full block
            for r in range(DB):
                js.append((I * DB + r, r * 128, QT - r * 128))  # diag
            nkb = len(js)
            ats = []
            for h in range(HL):
                qt = qts_tiles[I][h]
                s_tiles = [None] * nkb
                e_tiles = [None] * nkb

                def score(jj):
                    j, qoff, nq = js[jj]
                    s = psB.tile([128, QT], F32, name="s", tag="ps")
                    nc.tensor.matmul(s[:, 0:nq],
                                     kT_all[:, j * 128:(j + 1) * 128],
                                     qt[:, qoff:QT], start=True, stop=True)
                    s_tiles[jj] = s

                def sum_mm(jj):
                    j, qoff, nq = js[jj]
                    nc.tensor.matmul(sum_ps[0:1, qoff:QT], ones_col[:],
                                     e_tiles[jj][:, 0:nq], start=(jj == 0),
                                     stop=(jj == nkb - 1))
                    e_tiles[jj] = None

                out_ps = psA.tile([128, QT], F32, name="o", tag="psA")
                sum_ps = psA.tile([1, QT], F32, name="sum", tag="psA")
                score(0)
                if nkb > 1:
                    score(1)
                if nkb > 2:
                    score(2)
                for jj in range(nkb):
                    j, qoff, nq = js[jj]
                    e = expp.tile([128, QT], BF16, name="e", tag="e")
                    nc.scalar.activation(e[:, 0:nq], s_tiles[jj][:, 0:nq],
                                         AF.Exp, bias=0.0, scale=ISQ)
                    s_tiles[jj] = None
                    if j >= I * DB:  # diagonal boundary sub-block mask
                        nc.vector.tensor_tensor(out=e[:, 0:128],
                                                in0=e[:, 0:128],
                                                in1=mk_sb[:], op=ALU.mult)
                    e_tiles[jj] = e
                    nc.tensor.matmul(out_ps[:, qoff:QT], v_nat[:, j, :],
                                     e[:, 0:nq], start=(jj == 0),
                                     stop=(jj == nkb - 1))
                    if jj + 3 < nkb:
                        score(jj + 3)
                    if jj >= 2:
                        sum_mm(jj - 2)
                sum_mm(nkb - 2)
                sum_mm(nkb - 1)
                rec = recp.tile([1, QT], F32, name="rec", tag="rec")
                nc.vector.reciprocal(rec[:], sum_ps[0:1, :])
                rb = recbc.tile([128, QT], F32, name="rb", tag="rb")
                nc.gpsimd.partition_broadcast(rb[:], rec[:])
                a_t = apool.tile([128, QT], BF16, name="at", tag="at")
                nc.vector.tensor_tensor(out=a_t[:], in0=out_ps[:], in1=rb[:],
                                        op=ALU.mult)
                ats.append(a_t)
            at_tiles[I] = ats

        def emit_wo(I):
            ats = at_tiles[I]
            for t in range(DB):
                t_sl = slice(t * 128, (t + 1) * 128)
                y_sb = ysbp.tile([128, D], BF16, name="ysb", tag="ysb")
                for dc in range(D // 512):
                    y_ps = psY.tile([128, 512], F32, name="yps", tag="psY")
                    for hb in range(HL):
                        nc.tensor.matmul(y_ps[:], ats[hb][:, t_sl],
                                         wo_sb[:, hb, dc * 512:(dc + 1) * 512],
                                         start=(hb == 0), stop=(hb == HL - 1))
                    if dc % 2 == 0:
                        nc.scalar.activation(y_sb[:, dc * 512:(dc + 1) * 512],
                                             y_ps[:], AF.Copy, bias=0.0,
                                             scale=1.0)
                    else:
                        nc.vector.tensor_copy(
                            y_sb[:, dc * 512:(dc + 1) * 512], y_ps[:])
                row0 = I * QT + t * 128
                nc.sync.dma_start(out=y[row0:row0 + 128, :], in_=y_sb[:])

        # ---- main pipeline ----
        for I in range(NQ):
            hs_tiles = emit_hs_dma(I)
            if I > 0:
                emit_tbk(I - 1)
            emit_proj(I, hs_tiles)
            if I > 0:
                emit_attn(I - 1)
                emit_wo(I - 1)
            emit_transp(I)
        emit_tbk(NQ - 1)
        emit_attn(NQ - 1)
        emit_wo(NQ - 1)

    nc.compile()
    return nc


def host_inputs(hidden_states, wq, wk, wv, wo, position_ids,
                S=2048, D=4096, HL=4, QT=512, ncores=NCORES, mmdt="bf16"):
    """Shard + preprocess inputs -> per-core in_maps."""
    import ml_dtypes
    BF = ml_dtypes.bfloat16

    def cast(a):
        return np.ascontiguousarray(a).astype(BF)

    hs = np.asarray(hidden_states, np.float32)[0]
    hsT = cast(hs.T)  # [D, S]

    pos = np.asarray(position_ids)[0].astype(np.float32)
    inv_freq = (1.0 / (10000.0 ** (np.arange(0, 128, 2, dtype=np.float32)
                                   / 128.0)))
    freqs = pos[:, None] * inv_freq[None, :]          # [S, 64]
    emb = np.concatenate([freqs, freqs], axis=1)      # [S, 128]
    cosT = cast(np.cos(emb).T)
    sinT = cast(np.sin(emb).T)

    kk = np.arange(128)[:, None]
    qq = np.arange(128)[None, :]
    mkT = cast((kk <= qq).astype(np.float32))

    rotT = np.zeros((128, 128), np.float32)
    idx = np.arange(64)
    rotT[idx, idx + 64] = 1.0
    rotT[idx + 64, idx] = -1.0
    rotT = cast(rotT)

    wq = np.asarray(wq, np.float32)
    wk = np.asarray(wk, np.float32)
    wv = np.asarray(wv, np.float32)
    wo = np.asarray(wo, np.float32)

    in_maps = []
    qh = HL * 128
    for c in range(ncores):
        in_maps.append({
            "hsT": hsT,
            "wqT": cast(wq[c * qh:(c + 1) * qh, :].T),
            "wkT": cast(wk[c * 128:(c + 1) * 128, :].T),
            "wvT": cast(wv[c * 128:(c + 1) * 128, :].T),
            "woT": cast(wo[:, c * qh:(c + 1) * qh].T),
            "cosT": cosT, "sinT": sinT, "mkT": mkT, "rotT": rotT,
        })
    return in_maps


_NC_CACHE = {}
COMPUTE = "bf16"


def kernel(hidden_states, wq, wk, wv, wo, position_ids):
    B, S, D = hidden_states.shape
    in_maps = host_inputs(hidden_states, wq, wk, wv, wo, position_ids,
                          S=S, D=D, mmdt=COMPUTE)
    key = (S, D, COMPUTE)
    if key not in _NC_CACHE:
        _NC_CACHE[key] = build_nc(S=S, D=D)
    nc = _NC_CACHE[key]
    res = run_bass_kernel_spmd(nc, in_maps, core_ids=list(range(NCORES)),
                               trace=False)
    y = np.zeros((S, D), np.float64)
    for c in range(NCORES):
        y += res.results[c]["y"].astype(np.float64)
    return y.astype(np.float32)[None]
